# revision 7
# baseline (speedup 1.0000x reference)
"""CrossNetMix (DCN-V2 mixture-of-low-rank-experts) Trainium2 kernel.

Strategy: data-parallel over batch across 8 cores (2048 rows each), tensors
kept feature-major on chip ([d, b]) so every matmul contraction lands on SBUF
partitions. All matmul operands are bf16 (fp32 PSUM accumulation) — this
enables fast weight load (FWL) so LDWEIGHTS hides behind the matmul stream,
and halves HBM traffic. The host pre-packs weights/activations partition-major
so every DMA is contiguous per partition.

Per layer (fused), per 512-row batch chunk:
  g_rep = xi @ Wg4              -> gating, output replicated at partition
                                   bases 0/32/64/96 (for row-tiled broadcast)
  expg  = exp(g_rep)            -> ACT
  sums/rb                       -> partition-sum and 1/sum broadcast via tiny
                                   matmuls; wsb = expg * rb
  h1 = tanh(xi @ Vflat)         -> [er=512, b]
  h2 = tanh(blockdiag_C @ h1)   -> per-expert C folded into 128x128 pairs
  wb_mc = sel_mc.T @ wsb_rep    -> gate weights broadcast over ranks; 4
                                   row-tiled matmuls (tile_position) that can
                                   run concurrently in the PE array
  y  = h2 * wb
  mixed = Uflat.T @ y           -> [d, b]
  xi = x0 * (mixed + b) + xi    -> fused combine from PSUM

The three cross layers are emitted layer-outer / chunk-inner with per-chunk
persistent xi tiles, so the four batch chunks form independent pipelines and
the PE never waits on a single chunk's softmax/activation chain.
"""

import sys

import numpy as np
from ml_dtypes import bfloat16

if "/opt/trn_rl_repo" not in sys.path:
    sys.path.insert(0, "/opt/trn_rl_repo")

import concourse.bass as bass
import concourse.bacc as bacc
import concourse.mybir as mybir
from concourse.tile import TileContext
from concourse.bass_utils import run_bass_kernel_spmd

AF = mybir.ActivationFunctionType
OP = mybir.AluOpType
F32 = mybir.dt.float32
BF16 = mybir.dt.bfloat16

N_CROSS = 3
E = 8            # experts
D = 1024         # feature dim
R = 64           # low rank
B = 16384        # full batch
NCORES = 8
BC = B // NCORES  # rows per core
CHUNK = 512       # batch tile (matmul free dim)
NCHUNK = BC // CHUNK
P = 128
KC = D // P       # d-chunks
ER = E * R        # 512
MC = ER // P      # (e,r)-chunks


def _build():
    nc = bacc.Bacc(None)
    Xp = nc.declare_dram_parameter("Xp", [P, NCHUNK, KC, CHUNK], BF16, isOutput=False)
    Vp = nc.declare_dram_parameter("Vp", [P, N_CROSS, KC, ER], BF16, isOutput=False)
    Cp = nc.declare_dram_parameter("Cp", [P, N_CROSS, MC, P], BF16, isOutput=False)
    Up = nc.declare_dram_parameter("Up", [P, N_CROSS, MC, D], BF16, isOutput=False)
    Wgp = nc.declare_dram_parameter("Wgp", [P, KC, P], BF16, isOutput=False)
    SelWB = nc.declare_dram_parameter("SelWB", [P, P], BF16, isOutput=False)
    OnesC = nc.declare_dram_parameter("OnesC", [P, 1], BF16, isOutput=False)
    OnesR = nc.declare_dram_parameter("OnesR", [1, P], BF16, isOutput=False)
    Bp = nc.declare_dram_parameter("Bp", [P, N_CROSS, KC], F32, isOutput=False)
    OutT = nc.declare_dram_parameter("OutT", [NCHUNK, KC, P, CHUNK], BF16, isOutput=True)

    with TileContext(nc) as tc:
        with (
            tc.sbuf_pool(name="wpool", bufs=1) as wpool,
            tc.sbuf_pool(name="xpool", bufs=1) as xpool,
            tc.sbuf_pool(name="xipool", bufs=1) as xipool,
            tc.sbuf_pool(name="h1pool", bufs=8) as h1pool,
            tc.sbuf_pool(name="h2pool", bufs=3) as h2pool,
            tc.sbuf_pool(name="ypool", bufs=8) as ypool,
            tc.sbuf_pool(name="tpool", bufs=4) as tpool,
            tc.sbuf_pool(name="spool", bufs=3) as spool,
            tc.psum_pool(name="psmm", bufs=3) as psmm,
            tc.psum_pool(name="psu", bufs=2) as psu,
            tc.psum_pool(name="pswb", bufs=2) as pswb,
            tc.psum_pool(name="psg", bufs=1) as psg,
        ):
            # ---- resident weights / selectors ----
            wg_sb = wpool.tile([P, KC, P], BF16)
            nc.sync.dma_start(wg_sb, Wgp[:])
            selwb_sb = wpool.tile([P, P], BF16)
            nc.sync.dma_start(selwb_sb, SelWB[:])
            onesc_sb = wpool.tile([P, 1], BF16)
            nc.sync.dma_start(onesc_sb, OnesC[:])
            onesr_sb = wpool.tile([1, P], BF16)
            nc.sync.dma_start(onesr_sb, OnesR[:])
            b_sb = wpool.tile([P, N_CROSS, KC], F32)
            nc.sync.dma_start(b_sb, Bp[:])

            v_sb = wpool.tile([P, N_CROSS, KC, ER], BF16)
            u_sb = wpool.tile([P, N_CROSS, MC, D], BF16)
            c_sb = wpool.tile([P, N_CROSS, MC, P], BF16)

            # x0: split chunk 0 finely so layer-0 compute starts ASAP
            x0s = []
            for c in range(NCHUNK):
                t = xpool.tile([P, KC, CHUNK], BF16, tag=f"x0_{c}")
                x0s.append(t)
            for q in range(4):
                sl = slice(q * (KC // 4), (q + 1) * (KC // 4))
                nc.sync.dma_start(x0s[0][:, sl], Xp[:, 0, sl])
            # layer-0 V weights per kc (matches the accumulation order)
            for kc in range(KC):
                nc.sync.dma_start(v_sb[:, 0, kc, :], Vp[:, 0, kc, :])
            nc.sync.dma_start(c_sb[:, 0], Cp[:, 0])
            for c in range(1, NCHUNK):
                nc.sync.dma_start(x0s[c], Xp[:, c])
            for mc in range(MC):
                nc.sync.dma_start(u_sb[:, 0, mc, :], Up[:, 0, mc, :])
            for i in range(1, N_CROSS):
                for kc in range(KC):
                    nc.sync.dma_start(v_sb[:, i, kc, :], Vp[:, i, kc, :])
                nc.sync.dma_start(c_sb[:, i], Cp[:, i])
                for mc in range(MC):
                    nc.sync.dma_start(u_sb[:, i, mc, :], Up[:, i, mc, :])

            xis = [
                xipool.tile([P, KC, CHUNK], BF16, tag=f"xi_{c}", name=f"xi_{c}")
                for c in range(NCHUNK)
            ]

            # PE warm-up: HAM un-throttles only after ~3.4us of sustained PE
            # activity, and the input DMA feed takes ~13us to deliver the
            # first tiles. Spin dummy matmuls on a memset scratch tile so the
            # real matmuls start at full clock.
            wsc = wpool.tile([P, 256], BF16, name="warm_scratch")
            nc.vector.memset(wsc, 0.0)
            wps_ = psmm.tile([P, 256], F32, tag="mm", name="warm_ps")
            for _ in range(80):
                nc.tensor.matmul(wps_, wsc[:, 0:128], wsc, start=True, stop=True)

            def gate_chain(i, c):
                """Softmax gate chain for slot (i, c) up to the normalized
                weights wsb. Emitted one slot ahead so the tiny chain matmuls
                never stall the PE queue behind ACT/DVE latencies."""
                src = x0s[c] if i == 0 else xis[c]
                gps = psg.tile([P, CHUNK], F32, tag="g", name=f"gps_{i}_{c}")
                for kc in range(KC):
                    nc.tensor.matmul(
                        gps,
                        wg_sb[:, kc, :],
                        src[:, kc, :],
                        start=(kc == 0),
                        stop=(kc == KC - 1),
                    )
                expg = spool.tile([P, CHUNK], BF16, tag="expg", name=f"expg_{i}_{c}")
                nc.scalar.activation(expg, gps, AF.Exp)
                sums = psg.tile([1, CHUNK], F32, tag="g", name=f"sums_{i}_{c}")
                nc.tensor.matmul(sums, onesc_sb, expg, start=True, stop=True)
                rfast = spool.tile([1, CHUNK], F32, tag="rfast", name=f"rf_{i}_{c}")
                nc.vector.reciprocal_approx_fast(rfast, sums)
                rrow = spool.tile([1, CHUNK], BF16, tag="rrow", name=f"rr_{i}_{c}")
                nc.vector.tensor_copy(rrow, rfast)
                rb = psg.tile([P, CHUNK], F32, tag="g", name=f"rb_{i}_{c}")
                nc.tensor.matmul(rb, onesr_sb, rrow, start=True, stop=True)
                wsb = spool.tile([P, CHUNK], BF16, tag="wsb", name=f"wsb_{i}_{c}")
                nc.vector.tensor_tensor(wsb, expg, rb, OP.mult)
                return wsb

            slots = [(i, c) for i in range(N_CROSS) for c in range(NCHUNK)]
            wsb_next = gate_chain(*slots[0])
            for si, (i, c) in enumerate(slots):
                x0 = x0s[c]
                xi = xis[c]
                src = x0 if i == 0 else xi
                # gate weights broadcast over rank blocks: 4 row-tiled
                # matmuls in distinct row groups (can run concurrently).
                # wsb was produced during the previous slot, and the wb PSUM
                # banks were freed by the previous slot's C stage, so these
                # never wait.
                wsb = wsb_next
                wbs = []
                for mc in range(MC):
                    wbp = pswb.tile([P, CHUNK], F32, tag="wb", name=f"wb_{i}_{c}_{mc}")
                    nc.tensor.matmul(
                        wbp,
                        selwb_sb[32 * mc : 32 * mc + 8, :],
                        wsb[32 * mc : 32 * mc + 8, :],
                        start=True,
                        stop=True,
                        tile_position=(32 * mc, 0),
                    )
                    wbs.append(wbp)
                # ---- V stage: h1 = tanh(Vflat.T @ xi) ----
                h1s = []
                for mc in range(MC):
                    vps = psmm.tile([P, CHUNK], F32, tag="mm")
                    for kc in range(KC):
                        nc.tensor.matmul(
                            vps,
                            v_sb[:, i, kc, mc * P : (mc + 1) * P],
                            src[:, kc, :],
                            start=(kc == 0),
                            stop=(kc == KC - 1),
                        )
                    h1 = h1pool.tile([P, CHUNK], BF16, tag="h1")
                    nc.scalar.activation(h1, vps, AF.Tanh)
                    h1s.append(h1)
                # prefetch the next slot's gate chain while this slot's
                # C/U stages run
                if si + 1 < len(slots):
                    wsb_next = gate_chain(*slots[si + 1])
                # ---- C stage (block-diag expert pairs) + gate scale ----
                ys = []
                for mc in range(MC):
                    cps = psmm.tile([P, CHUNK], F32, tag="mm")
                    nc.tensor.matmul(
                        cps, c_sb[:, i, mc, :], h1s[mc], start=True, stop=True
                    )
                    h2 = h2pool.tile([P, CHUNK], BF16, tag="h2")
                    nc.scalar.activation(h2, cps, AF.Tanh)
                    y = ypool.tile([P, CHUNK], BF16, tag="y")
                    nc.vector.tensor_tensor(y, h2, wbs[mc], OP.mult)
                    ys.append(y)
                # ---- U stage + fused combine ----
                for dc in range(KC):
                    ups = psu.tile([P, CHUNK], F32, tag="u")
                    for mc in range(MC):
                        nc.tensor.matmul(
                            ups,
                            u_sb[:, i, mc, dc * P : (dc + 1) * P],
                            ys[mc],
                            start=(mc == 0),
                            stop=(mc == MC - 1),
                        )
                    tmp = tpool.tile([P, CHUNK], BF16, tag="tmp")
                    nc.vector.scalar_tensor_tensor(
                        tmp,
                        ups,
                        b_sb[:, i, dc : dc + 1],
                        x0[:, dc, :],
                        OP.add,
                        OP.mult,
                    )
                    eng = nc.gpsimd if dc % 2 == 0 else nc.vector
                    eng.tensor_tensor(
                        xi[:, dc, :], tmp, src[:, dc, :], OP.add
                    )
                    if i == N_CROSS - 1:
                        nc.sync.dma_start(OutT[c, dc], xi[:, dc, :])
    nc.compile()
    return nc


_CTX = {}


def _get_nc():
    if "nc" not in _CTX:
        _CTX["nc"] = _build()
    return _CTX["nc"]


def _prep_weights(U, V, C, Wg, b):
    f = np.float32
    U = np.asarray(U, dtype=f)
    V = np.asarray(V, dtype=f)
    C = np.asarray(C, dtype=f)
    Wg = np.asarray(Wg, dtype=f)
    b = np.asarray(b, dtype=f)
    # Vl[i, d, e*R+r] = V[i, e, d, r]; partition-major: Vp[p, i, kc, m]
    Vl = V.transpose(0, 2, 1, 3).reshape(N_CROSS, D, ER)
    Vp = np.ascontiguousarray(
        Vl.reshape(N_CROSS, KC, P, ER).transpose(2, 0, 1, 3)
    ).astype(bfloat16)
    # Ul[i, e*R+r, d] = U[i, e, d, r]; Up[p, i, mc, d]
    Ul = U.transpose(0, 1, 3, 2).reshape(N_CROSS, ER, D)
    Up = np.ascontiguousarray(
        Ul.reshape(N_CROSS, MC, P, D).transpose(2, 0, 1, 3)
    ).astype(bfloat16)
    # block-diagonal expert pairs for the C stage; Cp[p, i, mc, s]
    Cb = np.zeros((N_CROSS, MC, P, P), dtype=f)
    for i in range(N_CROSS):
        for m in range(MC):
            Cb[i, m, :R, :R] = C[i, 2 * m]
            Cb[i, m, R:, R:] = C[i, 2 * m + 1]
    Cp = np.ascontiguousarray(Cb.transpose(2, 0, 1, 3)).astype(bfloat16)
    # gating weight with output replicated at col offsets 0/32/64/96
    Wg4 = np.zeros((D, P), dtype=f)
    for j in range(4):
        Wg4[:, 32 * j : 32 * j + E] = Wg.T
    Wgp = np.ascontiguousarray(
        Wg4.reshape(KC, P, P).transpose(1, 0, 2)
    ).astype(bfloat16)
    # row-tiled gate-broadcast selectors: rows 32*mc+q -> cols j with
    # q == 2*mc + j//R
    SelWB = np.zeros((P, P), dtype=f)
    for mc in range(MC):
        for j in range(P):
            SelWB[32 * mc + 2 * mc + j // R, j] = 1.0
    SelWB = SelWB.astype(bfloat16)
    OnesC = np.zeros((P, 1), dtype=f)
    OnesC[:E, 0] = 1.0
    OnesC = OnesC.astype(bfloat16)
    OnesR = np.ones((1, P), dtype=f).astype(bfloat16)
    # Bp[p, i, kc] = b[i, kc*P + p]
    Bp = np.ascontiguousarray(b.reshape(N_CROSS, KC, P).transpose(2, 0, 1))
    return dict(
        Vp=Vp, Up=Up, Cp=Cp, Wgp=Wgp, SelWB=SelWB, OnesC=OnesC, OnesR=OnesR, Bp=Bp
    )


def kernel(x, U, V, C, Wg, b, _trace=False):
    nc = _get_nc()
    w = _prep_weights(U, V, C, Wg, b)
    xs = np.asarray(x, dtype=np.float32).reshape(NCORES, BC, D)
    in_maps = []
    for ci in range(NCORES):
        # Xp[p, c, kc, j] = x[c*CHUNK + j, kc*P + p]
        xc = xs[ci].reshape(NCHUNK, CHUNK, KC, P).transpose(3, 0, 2, 1)
        m = {"Xp": np.ascontiguousarray(xc).astype(bfloat16)}
        m.update(w)
        in_maps.append(m)
    res = run_bass_kernel_spmd(nc, in_maps, list(range(NCORES)), trace=_trace)
    kernel.last_result = res
    outs = []
    for ci in range(NCORES):
        o = np.asarray(res.results[ci]["OutT"]).astype(np.float32)
        # OutT[c, kc, p, j] -> [c*CHUNK + j, kc*P + p]
        outs.append(o.transpose(0, 3, 1, 2).reshape(BC, D))
    out = np.concatenate(outs, axis=0)
    return np.ascontiguousarray(out, dtype=np.float32)


# revision 10
# speedup vs baseline: 1.0366x; 1.0366x over previous
"""CrossNetMix (DCN-V2 mixture-of-low-rank-experts) Trainium2 kernel.

Strategy: data-parallel over batch across 8 cores (2048 rows each), tensors
kept feature-major on chip ([d, b]) so every matmul contraction lands on SBUF
partitions. All matmul operands are bf16 (fp32 PSUM accumulation) — this
enables fast weight load (FWL) so LDWEIGHTS hides behind the matmul stream,
and halves HBM traffic. The host pre-packs weights/activations partition-major
so every DMA is contiguous per partition.

Per layer (fused), per 512-row batch chunk:
  g_rep = xi @ Wg4              -> gating, output replicated at partition
                                   bases 0/32/64/96 (for row-tiled broadcast)
  expg  = exp(g_rep)            -> ACT
  sums/rb                       -> partition-sum and 1/sum broadcast via tiny
                                   matmuls; wsb = expg * rb
  h1 = tanh(xi @ Vflat)         -> [er=512, b]
  h2 = tanh(blockdiag_C @ h1)   -> per-expert C folded into 128x128 pairs
  wb_mc = sel_mc.T @ wsb_rep    -> gate weights broadcast over ranks; 4
                                   row-tiled matmuls (tile_position) that can
                                   run concurrently in the PE array
  y  = h2 * wb
  mixed = Uflat.T @ y           -> [d, b]
  xi = x0 * (mixed + b) + xi    -> fused combine from PSUM

The three cross layers are emitted layer-outer / chunk-inner with per-chunk
persistent xi tiles, so the four batch chunks form independent pipelines and
the PE never waits on a single chunk's softmax/activation chain.
"""

import sys

import numpy as np
from ml_dtypes import bfloat16

if "/opt/trn_rl_repo" not in sys.path:
    sys.path.insert(0, "/opt/trn_rl_repo")

import concourse.bass as bass
import concourse.bacc as bacc
import concourse.mybir as mybir
from concourse.tile import TileContext
from concourse.bass_utils import run_bass_kernel_spmd

AF = mybir.ActivationFunctionType
OP = mybir.AluOpType
F32 = mybir.dt.float32
BF16 = mybir.dt.bfloat16

N_CROSS = 3
E = 8            # experts
D = 1024         # feature dim
R = 64           # low rank
B = 16384        # full batch
NCORES = 8
BC = B // NCORES  # rows per core
CHUNK = 512       # batch tile (matmul free dim)
NCHUNK = BC // CHUNK
P = 128
KC = D // P       # d-chunks
ER = E * R        # 512
MC = ER // P      # (e,r)-chunks


def _build():
    nc = bacc.Bacc(None)
    Xp = nc.declare_dram_parameter("Xp", [P, NCHUNK, KC, CHUNK], BF16, isOutput=False)
    Vp = nc.declare_dram_parameter("Vp", [P, N_CROSS, KC, ER], BF16, isOutput=False)
    Cp = nc.declare_dram_parameter("Cp", [P, N_CROSS, MC, P], BF16, isOutput=False)
    Up = nc.declare_dram_parameter("Up", [P, N_CROSS, MC, D], BF16, isOutput=False)
    Wgp = nc.declare_dram_parameter("Wgp", [P, KC, P], BF16, isOutput=False)
    SelWB = nc.declare_dram_parameter("SelWB", [P, P], BF16, isOutput=False)
    OnesC = nc.declare_dram_parameter("OnesC", [P, 1], BF16, isOutput=False)
    OnesR = nc.declare_dram_parameter("OnesR", [1, P], BF16, isOutput=False)
    Bp = nc.declare_dram_parameter("Bp", [P, N_CROSS, KC], F32, isOutput=False)
    OutT = nc.declare_dram_parameter("OutT", [NCHUNK, KC, P, CHUNK], BF16, isOutput=True)

    with TileContext(nc) as tc:
        with (
            tc.sbuf_pool(name="wpool", bufs=1) as wpool,
            tc.sbuf_pool(name="xpool", bufs=1) as xpool,
            tc.sbuf_pool(name="xipool", bufs=1) as xipool,
            tc.sbuf_pool(name="h1pool", bufs=8) as h1pool,
            tc.sbuf_pool(name="h2pool", bufs=3) as h2pool,
            tc.sbuf_pool(name="ypool", bufs=8) as ypool,
            tc.sbuf_pool(name="tpool", bufs=4) as tpool,
            tc.sbuf_pool(name="spool", bufs=3) as spool,
            tc.psum_pool(name="psmm", bufs=3) as psmm,
            tc.psum_pool(name="psu", bufs=2) as psu,
            tc.psum_pool(name="pswb", bufs=2) as pswb,
            tc.psum_pool(name="psg", bufs=1) as psg,
        ):
            # ---- resident weights / selectors ----
            wg_sb = wpool.tile([P, KC, P], BF16)
            nc.sync.dma_start(wg_sb, Wgp[:])
            selwb_sb = wpool.tile([P, P], BF16)
            nc.sync.dma_start(selwb_sb, SelWB[:])
            onesc_sb = wpool.tile([P, 1], BF16)
            nc.sync.dma_start(onesc_sb, OnesC[:])
            onesr_sb = wpool.tile([1, P], BF16)
            nc.sync.dma_start(onesr_sb, OnesR[:])
            b_sb = wpool.tile([P, N_CROSS, KC], F32)
            nc.sync.dma_start(b_sb, Bp[:])

            v_sb = wpool.tile([P, N_CROSS, KC, ER], BF16)
            u_sb = wpool.tile([P, N_CROSS, MC, D], BF16)
            c_sb = wpool.tile([P, N_CROSS, MC, P], BF16)

            # x0: split chunk 0 finely so layer-0 compute starts ASAP
            x0s = []
            for c in range(NCHUNK):
                t = xpool.tile([P, KC, CHUNK], BF16, tag=f"x0_{c}")
                x0s.append(t)
            for q in range(4):
                sl = slice(q * (KC // 4), (q + 1) * (KC // 4))
                nc.sync.dma_start(x0s[0][:, sl], Xp[:, 0, sl])
            # layer-0 V weights per kc (matches the accumulation order)
            for kc in range(KC):
                nc.sync.dma_start(v_sb[:, 0, kc, :], Vp[:, 0, kc, :])
            nc.sync.dma_start(c_sb[:, 0], Cp[:, 0])
            for c in range(1, NCHUNK):
                nc.sync.dma_start(x0s[c], Xp[:, c])
            for mc in range(MC):
                nc.sync.dma_start(u_sb[:, 0, mc, :], Up[:, 0, mc, :])
            for i in range(1, N_CROSS):
                for kc in range(KC):
                    nc.sync.dma_start(v_sb[:, i, kc, :], Vp[:, i, kc, :])
                nc.sync.dma_start(c_sb[:, i], Cp[:, i])
                for mc in range(MC):
                    nc.sync.dma_start(u_sb[:, i, mc, :], Up[:, i, mc, :])

            xis = [
                xipool.tile([P, KC, CHUNK], BF16, tag=f"xi_{c}", name=f"xi_{c}")
                for c in range(NCHUNK)
            ]

            # PE warm-up: HAM un-throttles only after ~3.4us of sustained PE
            # activity, and the input DMA feed takes ~13us to deliver the
            # first tiles. Spin dummy matmuls on a memset scratch tile so the
            # real matmuls start at full clock.
            wsc = wpool.tile([P, 256], BF16, name="warm_scratch")
            nc.vector.memset(wsc, 0.0)
            wps_ = psmm.tile([P, 256], F32, tag="mm", name="warm_ps")
            for _ in range(30):
                nc.tensor.matmul(wps_, wsc[:, 0:128], wsc, start=True, stop=True)

            def gate_chain(i, c):
                """Softmax gate chain for slot (i, c) up to the normalized
                weights wsb. Emitted one slot ahead so the tiny chain matmuls
                never stall the PE queue behind ACT/DVE latencies."""
                src = x0s[c] if i == 0 else xis[c]
                gps = psg.tile([P, CHUNK], F32, tag="g", name=f"gps_{i}_{c}")
                for kc in range(KC):
                    nc.tensor.matmul(
                        gps,
                        wg_sb[:, kc, :],
                        src[:, kc, :],
                        start=(kc == 0),
                        stop=(kc == KC - 1),
                    )
                expg = spool.tile([P, CHUNK], BF16, tag="expg", name=f"expg_{i}_{c}")
                nc.scalar.activation(expg, gps, AF.Exp)
                sums = psg.tile([1, CHUNK], F32, tag="g", name=f"sums_{i}_{c}")
                nc.tensor.matmul(sums, onesc_sb, expg, start=True, stop=True)
                rfast = spool.tile([1, CHUNK], F32, tag="rfast", name=f"rf_{i}_{c}")
                nc.vector.reciprocal_approx_fast(rfast, sums)
                rrow = spool.tile([1, CHUNK], BF16, tag="rrow", name=f"rr_{i}_{c}")
                nc.vector.tensor_copy(rrow, rfast)
                rb = psg.tile([P, CHUNK], F32, tag="g", name=f"rb_{i}_{c}")
                nc.tensor.matmul(rb, onesr_sb, rrow, start=True, stop=True)
                wsb = spool.tile([P, CHUNK], BF16, tag="wsb", name=f"wsb_{i}_{c}")
                nc.vector.tensor_tensor(wsb, expg, rb, OP.mult)
                return wsb

            slots = [(i, c) for i in range(N_CROSS) for c in range(NCHUNK)]
            wsb_next = gate_chain(*slots[0])
            for si, (i, c) in enumerate(slots):
                x0 = x0s[c]
                xi = xis[c]
                src = x0 if i == 0 else xi
                # prefetch the NEXT slot's gate chain at the top of this slot:
                # its ACT/DVE ops clear within the first few matmuls, so the
                # chain never blocks this slot's tanh/evacuation traffic on
                # the engine FIFOs.
                wsb = wsb_next
                if si + 1 < len(slots):
                    wsb_next = gate_chain(*slots[si + 1])
                # gate weights broadcast over rank blocks: 4 row-tiled
                # matmuls in distinct row groups (can run concurrently).
                # wsb was produced during the previous slot, and the wb PSUM
                # banks were freed by the previous slot's C stage, so these
                # never wait.
                wbs = []
                for mc in range(MC):
                    wbp = pswb.tile([P, CHUNK], F32, tag="wb", name=f"wb_{i}_{c}_{mc}")
                    nc.tensor.matmul(
                        wbp,
                        selwb_sb[32 * mc : 32 * mc + 8, :],
                        wsb[32 * mc : 32 * mc + 8, :],
                        start=True,
                        stop=True,
                        tile_position=(32 * mc, 0),
                    )
                    wbs.append(wbp)
                # ---- V stage: h1 = tanh(Vflat.T @ xi) ----
                h1s = []
                for mc in range(MC):
                    vps = psmm.tile([P, CHUNK], F32, tag="mm")
                    for kc in range(KC):
                        nc.tensor.matmul(
                            vps,
                            v_sb[:, i, kc, mc * P : (mc + 1) * P],
                            src[:, kc, :],
                            start=(kc == 0),
                            stop=(kc == KC - 1),
                        )
                    h1 = h1pool.tile([P, CHUNK], BF16, tag="h1")
                    nc.scalar.activation(h1, vps, AF.Tanh)
                    h1s.append(h1)
                # ---- C stage (block-diag expert pairs) + gate scale ----
                ys = []
                for mc in range(MC):
                    cps = psmm.tile([P, CHUNK], F32, tag="mm")
                    nc.tensor.matmul(
                        cps, c_sb[:, i, mc, :], h1s[mc], start=True, stop=True
                    )
                    h2 = h2pool.tile([P, CHUNK], BF16, tag="h2")
                    nc.scalar.activation(h2, cps, AF.Tanh)
                    y = ypool.tile([P, CHUNK], BF16, tag="y")
                    nc.vector.tensor_tensor(y, h2, wbs[mc], OP.mult)
                    ys.append(y)
                # ---- U stage + fused combine ----
                for dc in range(KC):
                    ups = psu.tile([P, CHUNK], F32, tag="u")
                    for mc in range(MC):
                        nc.tensor.matmul(
                            ups,
                            u_sb[:, i, mc, dc * P : (dc + 1) * P],
                            ys[mc],
                            start=(mc == 0),
                            stop=(mc == MC - 1),
                        )
                    tmp = tpool.tile([P, CHUNK], BF16, tag="tmp")
                    nc.vector.scalar_tensor_tensor(
                        tmp,
                        ups,
                        b_sb[:, i, dc : dc + 1],
                        x0[:, dc, :],
                        OP.add,
                        OP.mult,
                    )
                    eng = nc.gpsimd if dc % 2 == 0 else nc.vector
                    eng.tensor_tensor(
                        xi[:, dc, :], tmp, src[:, dc, :], OP.add
                    )
                    if i == N_CROSS - 1:
                        nc.sync.dma_start(OutT[c, dc], xi[:, dc, :])
    nc.compile()
    return nc


_CTX = {}


def _get_nc():
    if "nc" not in _CTX:
        _CTX["nc"] = _build()
    return _CTX["nc"]


def _prep_weights(U, V, C, Wg, b):
    f = np.float32
    U = np.asarray(U, dtype=f)
    V = np.asarray(V, dtype=f)
    C = np.asarray(C, dtype=f)
    Wg = np.asarray(Wg, dtype=f)
    b = np.asarray(b, dtype=f)
    # Vl[i, d, e*R+r] = V[i, e, d, r]; partition-major: Vp[p, i, kc, m]
    Vl = V.transpose(0, 2, 1, 3).reshape(N_CROSS, D, ER)
    Vp = np.ascontiguousarray(
        Vl.reshape(N_CROSS, KC, P, ER).transpose(2, 0, 1, 3)
    ).astype(bfloat16)
    # Ul[i, e*R+r, d] = U[i, e, d, r]; Up[p, i, mc, d]
    Ul = U.transpose(0, 1, 3, 2).reshape(N_CROSS, ER, D)
    Up = np.ascontiguousarray(
        Ul.reshape(N_CROSS, MC, P, D).transpose(2, 0, 1, 3)
    ).astype(bfloat16)
    # block-diagonal expert pairs for the C stage; Cp[p, i, mc, s]
    Cb = np.zeros((N_CROSS, MC, P, P), dtype=f)
    for i in range(N_CROSS):
        for m in range(MC):
            Cb[i, m, :R, :R] = C[i, 2 * m]
            Cb[i, m, R:, R:] = C[i, 2 * m + 1]
    Cp = np.ascontiguousarray(Cb.transpose(2, 0, 1, 3)).astype(bfloat16)
    # gating weight with output replicated at col offsets 0/32/64/96
    Wg4 = np.zeros((D, P), dtype=f)
    for j in range(4):
        Wg4[:, 32 * j : 32 * j + E] = Wg.T
    Wgp = np.ascontiguousarray(
        Wg4.reshape(KC, P, P).transpose(1, 0, 2)
    ).astype(bfloat16)
    # row-tiled gate-broadcast selectors: rows 32*mc+q -> cols j with
    # q == 2*mc + j//R
    SelWB = np.zeros((P, P), dtype=f)
    for mc in range(MC):
        for j in range(P):
            SelWB[32 * mc + 2 * mc + j // R, j] = 1.0
    SelWB = SelWB.astype(bfloat16)
    OnesC = np.zeros((P, 1), dtype=f)
    OnesC[:E, 0] = 1.0
    OnesC = OnesC.astype(bfloat16)
    OnesR = np.ones((1, P), dtype=f).astype(bfloat16)
    # Bp[p, i, kc] = b[i, kc*P + p]
    Bp = np.ascontiguousarray(b.reshape(N_CROSS, KC, P).transpose(2, 0, 1))
    return dict(
        Vp=Vp, Up=Up, Cp=Cp, Wgp=Wgp, SelWB=SelWB, OnesC=OnesC, OnesR=OnesR, Bp=Bp
    )


def kernel(x, U, V, C, Wg, b, _trace=False):
    nc = _get_nc()
    w = _prep_weights(U, V, C, Wg, b)
    xs = np.asarray(x, dtype=np.float32).reshape(NCORES, BC, D)
    in_maps = []
    for ci in range(NCORES):
        # Xp[p, c, kc, j] = x[c*CHUNK + j, kc*P + p]
        xc = xs[ci].reshape(NCHUNK, CHUNK, KC, P).transpose(3, 0, 2, 1)
        m = {"Xp": np.ascontiguousarray(xc).astype(bfloat16)}
        m.update(w)
        in_maps.append(m)
    res = run_bass_kernel_spmd(nc, in_maps, list(range(NCORES)), trace=_trace)
    kernel.last_result = res
    outs = []
    for ci in range(NCORES):
        o = np.asarray(res.results[ci]["OutT"]).astype(np.float32)
        # OutT[c, kc, p, j] -> [c*CHUNK + j, kc*P + p]
        outs.append(o.transpose(0, 3, 1, 2).reshape(BC, D))
    out = np.concatenate(outs, axis=0)
    return np.ascontiguousarray(out, dtype=np.float32)
